# revision 1
# baseline (speedup 1.0000x reference)
"""nn_DTW kernel for 8 Trainium2 NeuronCores (batch data-parallel).

See _build_cfg for the device program; the host does the sequential
backtrack pointer-chase and the final logsumexp combine.
"""

from contextlib import ExitStack

import concourse.bass as bass
import concourse.bacc as bacc
import concourse.tile as tile
from concourse import mybir
from concourse.masks import make_identity

F32 = mybir.dt.float32
AX = mybir.AxisListType
OP = mybir.AluOpType
ACT = mybir.ActivationFunctionType

BIG = 1.0e30


def _build_cfg(B=8, N=512, M=512, D=256, S=16, W=32, R=8, PART=128):
    assert S * W == M and N % R == 0
    P = S * B
    assert P <= PART
    NT = (N + PART - 1) // PART
    MT = (M + PART - 1) // PART
    DB = (D + PART - 1) // PART
    PN = min(PART, N)
    PD = min(PART, D)
    NSTEP = N // R
    T_TOT = NSTEP + S - 1
    SLOTS = N + R * S
    SLOT = W + 1

    nc = bacc.Bacc("TRN2", target_bir_lowering=False, debug=False)

    x_in = nc.dram_tensor("x", [B, N, D], F32, kind="ExternalInput").ap()
    y_in = nc.dram_tensor("y", [B, M, D], F32, kind="ExternalInput").ap()
    tc_out = nc.dram_tensor("tc_out", [P, SLOTS, SLOT], F32, kind="ExternalOutput").ap()
    neg_out = nc.dram_tensor("neg_out", [B, 1], F32, kind="ExternalOutput").ap()
    cost_stage = nc.dram_tensor("cost_stage", [NT, B, PN, M], F32).ap()

    with tile.TileContext(nc) as tcx, ExitStack() as ctx:
        const = ctx.enter_context(tcx.tile_pool(name="const", bufs=1))
        ident = const.tile([PART, PART], F32)
        make_identity(nc, ident[:])
        oneh = const.tile([PN, B, B], F32)
        nc.vector.memset(oneh[:], 0.0)
        for b_ in range(B):
            nc.vector.memset(oneh[:, b_, b_:b_ + 1], 1.0)
        big_m0 = const.tile([P, W], F32)
        nc.vector.memset(big_m0[:], BIG)
        shift8 = const.tile([PART, PART], F32)
        nc.gpsimd.memset(shift8[:], 0.0)
        nc.gpsimd.affine_select(
            out=shift8[:], in_=shift8[:], compare_op=OP.not_equal, fill=1.0,
            base=B, pattern=[[-1, PART]], channel_multiplier=1,
        )
        bigrow = const.tile([1, PART], F32)
        nc.vector.memset(bigrow[:], 0.0)
        nc.vector.memset(bigrow[0:1, 0:B], BIG)
        onesR = const.tile([1, R], F32)
        nc.vector.memset(onesR[:], 1.0)

        strip = ctx.enter_context(tcx.tile_pool(name="strip", bufs=1))
        tc_strip = strip.tile([P, SLOTS, SLOT], F32)
        nc.gpsimd.memset(tc_strip[:, :, :], BIG)

        # persistent transposed operands + per-batch scales
        oper = ctx.enter_context(tcx.tile_pool(name="oper", bufs=1))
        xTall = oper.tile([PD, B, DB, N], F32)
        ynTall = oper.tile([PD, B, DB, M], F32)
        xrn_all = oper.tile([PN, B, NT], F32)

        # stage-B pools created up-front so B0 can interleave with stage A
        stage = ctx.enter_context(tcx.tile_pool(name="stage", bufs=3))
        neg_pool = ctx.enter_context(tcx.tile_pool(name="negp", bufs=1))
        ps_c = ctx.enter_context(tcx.tile_pool(name="ps_c", bufs=3, space="PSUM"))
        ps_neg = ctx.enter_context(tcx.tile_pool(name="ps_neg", bufs=1, space="PSUM"))
        ngb = ps_neg.tile([B, M], F32, tag="ngb", bufs=1)

        def emit_stageB_batch(nt, b):
            rows = min(PART, N - nt * PART)
            psc = ps_c.tile([PN, M], F32, tag="psc", name=f"psc_{nt}_{b}")
            for db in range(DB):
                dcols = min(PART, D - db * PART)
                nc.tensor.matmul(
                    psc[:rows, :],
                    xTall[:dcols, b, db, nt * PART:nt * PART + rows],
                    ynTall[:dcols, b, db, :],
                    start=(db == 0), stop=(db == DB - 1),
                )
            cn = stage.tile([PN, M], F32, tag="cn", name=f"cn_{nt}_{b}")
            nc.scalar.activation(cn[:rows], psc[:rows], ACT.Copy,
                                 scale=xrn_all[:rows, b, nt:nt + 1], bias=1.0)
            nc.tensor.matmul(
                ngb[:, :],
                oneh[:rows, b, :],
                cn[:rows, :],
                start=(nt == 0 and b == 0),
                stop=(nt == NT - 1 and b == B - 1),
                skip_group_check=True,
            )
            heng = nc.scalar if b % 2 == 0 else nc.sync
            heng.dma_start(out=cost_stage[nt, b], in_=cn[:rows, :])

        def emit_hop2_part(nt, quarter):
            rows = min(PART, N - nt * PART)
            for s in range(quarter * S // 4, (quarter + 1) * S // 4):
                src = cost_stage[nt, :, :, s * W:(s + 1) * W]
                eng = nc.sync if s % 2 == 0 else nc.scalar
                eng.dma_start(
                    out=tc_strip[s * B:s * B + B,
                                 R * s + nt * PART:R * s + nt * PART + rows,
                                 1:SLOT],
                    in_=src)

        def emit_hop2(nt):
            rows = min(PART, N - nt * PART)
            for s in range(S):
                src = cost_stage[nt, :, :, s * W:(s + 1) * W]
                eng = nc.sync if s % 2 == 0 else nc.scalar
                eng.dma_start(
                    out=tc_strip[s * B:s * B + B,
                                 R * s + nt * PART:R * s + nt * PART + rows,
                                 1:SLOT],
                    in_=src)

        def emit_stageB(nt):
            for b in range(B):
                emit_stageB_batch(nt, b)
            emit_hop2(nt)

        # ---------------- Stage A: loads, norms, transposes ----------------
        with ExitStack() as ctxA:
            xy = ctxA.enter_context(tcx.tile_pool(name="xy", bufs=2))
            nrm = ctxA.enter_context(tcx.tile_pool(name="nrm", bufs=3))
            ps_t = ctxA.enter_context(tcx.tile_pool(name="ps_t", bufs=2, space="PSUM"))

            for b in range(B):
                y_all = xy.tile([PN, MT, D], F32, tag="ldy")
                nc.sync.dma_start(
                    out=y_all[:, :, :],
                    in_=y_in[b].rearrange("(t n) d -> n t d", t=MT))
                x_all = xy.tile([PN, NT, D], F32, tag="ldx")
                nc.sync.dma_start(
                    out=x_all[:, :, :],
                    in_=x_in[b].rearrange("(t n) d -> n t d", t=NT))

                ps_y = [ps_t.tile([PD, M], F32, tag=f"pstr{db}", name=f"psy{db}_{b}")
                        for db in range(DB)]
                for mt in range(MT):
                    rows = min(PART, M - mt * PART)
                    yt = y_all[:rows, mt, :]
                    sq = xy.tile([PART, D], F32, tag="sq")
                    s2 = nrm.tile([PART, 1], F32, tag="s2")
                    nc.scalar.activation(sq[:rows], yt, ACT.Square, accum_out=s2[:rows])
                    nrm_t = nrm.tile([PART, 1], F32, tag="nrm")
                    nc.scalar.activation(nrm_t[:rows], s2[:rows], ACT.Sqrt)
                    rn = nrm.tile([PART, 1], F32, tag="rn")
                    nc.vector.reciprocal(rn[:rows], nrm_t[:rows])
                    yn = xy.tile([PART, D], F32, tag="yn")
                    nc.vector.tensor_scalar_mul(yn[:rows], yt, rn[:rows])
                    for db in range(DB):
                        dcols = min(PART, D - db * PART)
                        nc.tensor.transpose(
                            ps_y[db][:dcols, mt * PART:mt * PART + rows],
                            yn[:rows, db * PART:db * PART + dcols],
                            ident[:rows, :rows])
                for db in range(DB):
                    nc.scalar.copy(ynTall[:, b, db, :], ps_y[db][:, :])

                ps_x = [ps_t.tile([PD, N], F32, tag=f"pstr{db}", name=f"psx{db}_{b}")
                        for db in range(DB)]
                for nt in range(NT):
                    rows = min(PART, N - nt * PART)
                    xt = x_all[:rows, nt, :]
                    sq = xy.tile([PART, D], F32, tag="sq")
                    s2 = nrm.tile([PART, 1], F32, tag="s2")
                    nc.scalar.activation(sq[:rows], xt, ACT.Square, accum_out=s2[:rows])
                    nrm_t = nrm.tile([PART, 1], F32, tag="nrm")
                    nc.scalar.activation(nrm_t[:rows], s2[:rows], ACT.Sqrt)
                    rn = nrm.tile([PART, 1], F32, tag="rn")
                    nc.vector.reciprocal(rn[:rows], nrm_t[:rows])
                    nc.vector.tensor_scalar_mul(xrn_all[:rows, b, nt:nt + 1],
                                                rn[:rows], -1.0)
                    for db in range(DB):
                        dcols = min(PART, D - db * PART)
                        nc.tensor.transpose(
                            ps_x[db][:dcols, nt * PART:nt * PART + rows],
                            xt[:, db * PART:db * PART + dcols],
                            ident[:rows, :rows])
                for db in range(DB):
                    nc.scalar.copy(xTall[:, b, db, :], ps_x[db][:, :])
                # interleave the first row-block's cost work for this batch
                emit_stageB_batch(0, b)
            emit_hop2(0)

        # ---------------- Stage C: skew-R DTW wavefront ----------------
        ps_carry = ctx.enter_context(tcx.tile_pool(name="ps_cr", bufs=1, space="PSUM"))
        mpool = ctx.enter_context(tcx.tile_pool(name="mpool", bufs=8))

        NCARRY = 4
        carry_tiles = [
            ps_carry.tile([P, R], F32, tag=f"cr{i}", name=f"carry{i}")
            for i in range(NCARRY)
        ]

        def emit_carry(U, c0, c1):
            base = R * U
            bnd = min(S - 1, U + 1) * B
            cps = carry_tiles[(U + 1) % NCARRY]
            nc.tensor.matmul(
                cps[0:bnd + B, c0:c1],
                shift8[0:bnd, 0:bnd + B],
                tc_strip[0:bnd, base + c0:base + c1, SLOT - 1:SLOT],
                start=True, stop=False, skip_group_check=True,
            )
            nc.tensor.matmul(
                cps[0:bnd + B, c0:c1],
                bigrow[0:1, 0:bnd + B],
                onesR[0:1, 0:c1 - c0],
                start=False, stop=True, skip_group_check=True,
            )
            return cps

        state = {"prev_carry": None, "out_lo": 0}

        def emit_stageC(U0, U1, sprinkle=None):
            for U in range(U0, U1):
                if sprinkle:
                    for off, fn in sprinkle:
                        if U == U0 + off:
                            fn()
                smax = min(S - 1, U)
                phi = (smax + 1) * B
                base = R * U

                for k in range(R):
                    q = base + k
                    if U == 0 and k == 0:
                        m_ap = big_m0[0:phi, :]
                    else:
                        mt_ = mpool.tile([P, W], F32, tag="m", name=f"m_{U}_{k}")
                        nc.vector.tensor_tensor(
                            mt_[0:phi, :],
                            tc_strip[0:phi, q - 1, 0:W],
                            tc_strip[0:phi, q - 1, 1:SLOT],
                            OP.min,
                        )
                        m_ap = mt_[0:phi, :]

                    if U == 0:
                        init = 0.0 if k == 0 else BIG
                    else:
                        init = state["prev_carry"][0:phi, k:k + 1]
                    nc.vector.tensor_tensor_scan(
                        tc_strip[0:phi, q, 1:SLOT],
                        m_ap,
                        tc_strip[0:phi, q, 1:SLOT],
                        init,
                        OP.min,
                        OP.add,
                    )
                    if U + 1 < T_TOT:
                        if k == R - 2:
                            state["cps"] = emit_carry(U, 0, R - 1)
                        elif k == R - 1:
                            cps = emit_carry(U, R - 1, R)
                            bnd = min(S - 1, U + 1) * B
                            nc.scalar.copy(
                                tc_strip[0:bnd + B, base + R:base + 2 * R, 0:1],
                                cps[0:bnd + B, 0:R])
                            state["prev_carry"] = state["cps"]
                # stream finished slots out every 16 supersteps
                if (U + 1) % 8 == 0 and U + 1 < T_TOT:
                    lo, hi = state["out_lo"], (U + 1) * R
                    nc.sync.dma_start(out=tc_out[:, lo:hi, :],
                                      in_=tc_strip[:, lo:hi, :])
                    state["out_lo"] = hi

        # Interleave stage-B blocks with stage-C chunks so each engine's
        # in-order queue pipelines across stages. C-chunk for block nt covers
        # supersteps [nt*PART/R, (nt+1)*PART/R).
        UPB = PART // R                 # supersteps per row-block
        for nt in range(1, NT):
            # spread block nt's batches across chunk nt-1's supersteps
            spr = [(min(2 * b_, UPB - 4), (lambda n_, bb: lambda: emit_stageB_batch(n_, bb))(nt, b_))
                   for b_ in range(B)]
            for qi in range(4):
                spr.append((UPB - 3 + min(qi, 2),
                            (lambda n_, q_: lambda: emit_hop2_part(n_, q_))(nt, qi)))
            emit_stageC((nt - 1) * UPB, nt * UPB, sprinkle=spr)
        emit_stageC((NT - 1) * UPB, T_TOT)

        # neg = logsumexp over m (emitted last; only needed at the end)
        negsum = neg_pool.tile([B, M], F32)
        nc.scalar.copy(negsum[:, :], ngb[:, :])
        mx = neg_pool.tile([B, 1], F32)
        nc.vector.reduce_max(mx[:], negsum[:], AX.X)
        sh = neg_pool.tile([B, M], F32)
        nc.vector.tensor_scalar(sh[:], negsum[:], mx[:], None, OP.subtract)
        ex = neg_pool.tile([B, M], F32)
        esum = neg_pool.tile([B, 1], F32)
        nc.scalar.activation(ex[:], sh[:], ACT.Exp, accum_out=esum[:])
        lg = neg_pool.tile([B, 1], F32)
        nc.scalar.activation(lg[:], esum[:], ACT.Ln)
        negv = neg_pool.tile([B, 1], F32)
        nc.vector.tensor_add(negv[:], lg[:], mx[:])
        nc.sync.dma_start(out=neg_out[:, :], in_=negv[:])

        lo = state["out_lo"]
        nc.sync.dma_start(out=tc_out[:, lo:SLOTS, :], in_=tc_strip[:, lo:SLOTS, :])

    nc.compile()
    return nc


# ---------------------------------------------------------------------------
# Host-side driver: sharding, run, unskew, backtrack walk, final loss
# ---------------------------------------------------------------------------
import numpy as np

B_TOT, N_G, M_G, D_G = 64, 512, 512, 256
N_CORES = 8
B_LOC = B_TOT // N_CORES
S_G, W_G, R_G = 16, 32, 8
P_G = S_G * B_LOC
SLOTS_G = N_G + R_G * S_G
SLOT_G = W_G + 1

_NC_CACHE = {}


def _get_nc():
    if "nc" not in _NC_CACHE:
        _NC_CACHE["nc"] = _build_cfg(B=B_LOC, N=N_G, M=M_G, D=D_G,
                                     S=S_G, W=W_G, R=R_G)
    return _NC_CACHE["nc"]


def _unskew(tc_skew):
    tc = np.empty((B_LOC, N_G, M_G), np.float32)
    for s in range(S_G):
        for b in range(B_LOC):
            tc[b, :, s * W_G:(s + 1) * W_G] = \
                tc_skew[s * B_LOC + b, R_G * s:R_G * s + N_G, 1:SLOT_G]
    return tc


def _host_finish(tc, x, y, neg):
    """Backtrack walk on the device tc + pos logsumexp (host side)."""
    Bt, Nn, Mm = tc.shape
    eps = 1e-8
    xn = x / np.maximum(np.linalg.norm(x, axis=-1, keepdims=True), eps)
    yn = y / np.maximum(np.linalg.norm(y, axis=-1, keepdims=True), eps)
    bidx = np.arange(Bt)
    i = np.full(Bt, Nn - 1, np.int64)
    j = np.full(Bt, Mm - 1, np.int64)
    Is, Js, Vs = [i.copy()], [j.copy()], [np.ones(Bt, bool)]
    active = (i > 0) & (j > 0)
    while active.any():
        a = tc[bidx, np.maximum(i - 1, 0), np.maximum(j - 1, 0)]
        bb = tc[bidx, np.maximum(i - 1, 0), j]
        c = tc[bidx, i, np.maximum(j - 1, 0)]
        diag = (a <= bb) & (a <= c)
        up = (~diag) & (bb <= c)
        ni = np.where(diag | up, i - 1, i)
        nj = np.where(diag | (~up), j - 1, j)
        i = np.where(active, ni, i)
        j = np.where(active, nj, j)
        Is.append(i.copy())
        Js.append(j.copy())
        Vs.append(active.copy())
        active = (i > 0) & (j > 0)
    at00 = (i == 0) & (j == 0)
    Is.append(np.zeros(Bt, np.int64))
    Js.append(np.zeros(Bt, np.int64))
    Vs.append(~at00)

    IS = np.stack(Is, 1)
    JS = np.stack(Js, 1)
    VS = np.stack(Vs, 1)
    costs = 1.0 - np.einsum("bld,bld->bl",
                            xn[bidx[:, None], IS], yn[bidx[:, None], JS])
    colsum = np.zeros((Bt, Mm), np.float32)
    np.add.at(colsum, (bidx[:, None], JS),
              np.where(VS, costs, 0.0).astype(np.float32))
    mxv = colsum.max(axis=1, keepdims=True)
    pos = (mxv + np.log(np.sum(np.exp(colsum - mxv),
                               axis=1, keepdims=True))).squeeze(1)
    return (pos.astype(np.float32) - neg).astype(np.float32)


def run_device(x, y, **kw):
    from concourse import bass_utils

    nc = _get_nc()
    in_maps = [
        {"x": np.ascontiguousarray(x[c * B_LOC:(c + 1) * B_LOC]),
         "y": np.ascontiguousarray(y[c * B_LOC:(c + 1) * B_LOC])}
        for c in range(N_CORES)
    ]
    res = bass_utils.run_bass_kernel_spmd(nc, in_maps, list(range(N_CORES)), **kw)
    tc = np.empty((B_TOT, N_G, M_G), np.float32)
    neg = np.empty(B_TOT, np.float32)
    for c in range(N_CORES):
        out = res.results[c]
        tc[c * B_LOC:(c + 1) * B_LOC] = _unskew(out["tc_out"])
        neg[c * B_LOC:(c + 1) * B_LOC] = out["neg_out"].reshape(B_LOC)
    return tc, neg, res


def kernel(x, y):
    x = np.asarray(x, dtype=np.float32)
    y = np.asarray(y, dtype=np.float32)
    tc, neg, _ = run_device(x, y)
    return _host_finish(tc, x, y, neg)



# revision 23
# speedup vs baseline: 1.2125x; 1.2125x over previous
"""nn_DTW kernel for 8 Trainium2 NeuronCores (batch data-parallel).

Device computes the cosine-cost matrix (bf16 matmuls) and the full DTW
cumulative table via a skewed column-strip wavefront on the Vector engine;
the host does input layout prep (normalize/transpose/bf16 cast), the
sequential backtrack pointer-chase, and the final logsumexp combine.

Wavefront layout: M=512 columns split into S=16 strips of W=32; partition
p = s*B + b holds strip s of batch b. tc_strip[p, 1+q, 0:33] holds row
(q - R*s) of the DTW table (position 0 is a BIG border column, positions
1..32 the values). Strips are skewed by R=8 rows so a single scan
instruction advances one row of every strip. The cross-strip (left
neighbor) dependency enters through the scan's `initial` operand:
initial = min(left_last[row i], left_last[row i-1]), computed as a small
DVE pair-min of last columns each superstep, then shifted one strip down
(+B partitions) by a PE matmul into PSUM.
"""

from contextlib import ExitStack

import concourse.bass as bass
import concourse.bacc as bacc
import concourse.tile as tile
from concourse import mybir

F32 = mybir.dt.float32
BF16 = mybir.dt.bfloat16
AX = mybir.AxisListType
OP = mybir.AluOpType
ACT = mybir.ActivationFunctionType

BIG = 1.0e30

_COMPUTE_OPS = (
    "TensorTensor", "TensorScalarPtr", "TensorReduce", "TensorCopy",
    "Activation", "Matmult", "Memset", "Copy", "TensorScalarAffineSelect",
    "ISA", "Reciprocal", "Iota", "Shift", "MaxIndex", "MatchValueIndex",
)


def _relax_same_engine_sems(nc):
    """Drop semaphore waits that only re-state same-engine program order.

    Each engine executes its queue in order, so a compute instruction never
    needs to wait on its own engine's completion semaphore: every prior
    same-engine instruction has fully executed (and its SBUF writes retired)
    before the next one starts. The tile scheduler still emits those waits;
    removing them eliminates a ~100ns sem-propagation stall between
    back-to-back dependent ops on the same engine. Cross-engine and DMA
    waits are preserved untouched.
    """
    fn = nc.m.functions[0]
    updaters = {}
    for bb in fn.blocks:
        for inst in bb.instructions:
            si = inst.sync_info
            if si is None:
                continue
            is_compute = inst.opcode in _COMPUTE_OPS
            for u in si.on_update:
                if u.sync_type != "semaphore":
                    continue
                tag = (inst.engine, is_compute and u.update_mode == "sem-inc")
                updaters.setdefault(u.ant_name, set()).add(tag)
    own_sem = {}
    for name, tags in updaters.items():
        if len(tags) == 1:
            (eng, ok), = tags
            if ok:
                own_sem.setdefault(eng, set()).add(name)
    ndrop = 0
    prior = {}
    for bb in fn.blocks:
        for inst in bb.instructions:
            si = inst.sync_info
            if si is None:
                continue
            eng = inst.engine
            mine = own_sem.get(eng, ())
            if inst.opcode in _COMPUTE_OPS and si.on_wait:
                keep = []
                for w in si.on_wait:
                    if (w.sync_type == "semaphore"
                            and w.wait_mode == "sem-ge-imm"
                            and w.ant_name in mine):
                        assert w.wait_value <= prior.get((eng, w.ant_name), 0), (
                            f"{inst.name}: wait {w.ant_name}>={w.wait_value} "
                            f"not implied by order "
                            f"({prior.get((eng, w.ant_name), 0)} prior)")
                        ndrop += 1
                    else:
                        keep.append(w)
                if len(keep) != len(si.on_wait):
                    si.on_wait.clear()
                    for w in keep:
                        si.on_wait.append(w)
            for u in si.on_update:
                if (u.sync_type == "semaphore" and u.ant_name in mine
                        and u.update_mode == "sem-inc"):
                    k = (eng, u.ant_name)
                    prior[k] = prior.get(k, 0) + u.update_value
    return ndrop


def _build_cfg(B=8, N=512, M=512, D=256, S=16, W=32, R=8, PART=128):
    assert S * W == M and N % R == 0
    P = S * B
    assert P <= PART
    NT = (N + PART - 1) // PART
    DB = (D + PART - 1) // PART
    PN = min(PART, N)
    PD = min(PART, D)
    NSTEP = N // R
    T_TOT = NSTEP + S - 1
    SLOTS = N + R * S
    SLOT = W + 1
    UPB = PART // R            # supersteps per row-block

    nc = bacc.Bacc("TRN2", target_bir_lowering=False, debug=False)

    xT_in = nc.dram_tensor("xT", [PD, B, DB, N], BF16, kind="ExternalInput").ap()
    ynT_in = nc.dram_tensor("ynT", [PD, B, DB, M], BF16, kind="ExternalInput").ap()
    xrn_in = nc.dram_tensor("xrn", [PN, B, NT], F32, kind="ExternalInput").ap()
    tc_out = nc.dram_tensor("tc_out", [P, SLOTS, SLOT], F32, kind="ExternalOutput").ap()
    neg_out = nc.dram_tensor("neg_out", [B, 1], F32, kind="ExternalOutput").ap()
    cost_stage = nc.dram_tensor("cost_stage", [NT, B, PN, M], BF16).ap()

    with tile.TileContext(nc) as tcx, ExitStack() as ctx:
        const = ctx.enter_context(tcx.tile_pool(name="const", bufs=1))
        shift8 = const.tile([PART, PART], F32)
        nc.gpsimd.memset(shift8[:], 0.0)
        nc.gpsimd.affine_select(
            out=shift8[:], in_=shift8[:], compare_op=OP.not_equal, fill=1.0,
            base=B, pattern=[[-1, PART]], channel_multiplier=1,
        )
        bigrow = const.tile([1, PART], F32)
        nc.gpsimd.memset(bigrow[:], 0.0)
        nc.gpsimd.memset(bigrow[0:1, 0:B], BIG)
        onesR = const.tile([1, R], F32)
        nc.gpsimd.memset(onesR[:], 1.0)
        oneh = const.tile([PN, B, B], BF16)
        nc.gpsimd.memset(oneh[:], 0.0)
        for b_ in range(B):
            nc.gpsimd.memset(oneh[:, b_, b_:b_ + 1], 1.0)

        bigpad = const.tile([B, 1, SLOT], F32)
        nc.gpsimd.memset(bigpad[:], BIG)
        zcost = const.tile([B, R * S, W], BF16)
        nc.gpsimd.memset(zcost[:], 0.0)

        strip = ctx.enter_context(tcx.tile_pool(name="strip", bufs=1))
        # physical slot 0 = BIG border; logical slot q lives at physical q+1
        tc_strip = strip.tile([P, 1 + SLOTS, SLOT], F32)
        cost_strip = strip.tile([P, SLOTS, W], BF16)
        # BIG borders only where the wavefront reads. Engine ops must start
        # at partition 0, so strip-local inits go through small DMAs instead
        # of memsets.
        nc.gpsimd.memset(tc_strip[:, 0:1, :], BIG)          # dummy slot
        nc.gpsimd.memset(tc_strip[:, :, 0:1], BIG)          # position-0 col
        for s in range(1, S):
            # strip s's row -1 (logical slot R*s-1 -> physical R*s)
            _Q0 = [nc.sync, nc.scalar, nc.gpsimd][s % 3]
            _Q0.dma_start(out=tc_strip[s * B:(s + 1) * B, R * s:R * s + 1, :],
                          in_=bigpad[:, :, :])
        for s in range(S):
            # finished strips keep scanning past their last row; zero cost
            # there keeps that junk finite (it is never read back as data,
            # but the next-strip carry matmul must not see NaNs).
            npad = R * (S - s)
            _Q0 = [nc.sync, nc.scalar, nc.gpsimd][s % 3]
            _Q0.dma_start(out=cost_strip[s * B:(s + 1) * B,
                                         R * s + N:SLOTS, :],
                          in_=zcost[:, 0:npad, :])

        oper = ctx.enter_context(tcx.tile_pool(name="oper", bufs=1))
        xTall = oper.tile([PD, B, DB, N], BF16)
        ynTall = oper.tile([PD, B, DB, M], BF16)
        xrn_all = oper.tile([PN, B, NT], F32)
        nc.sync.dma_start(out=xrn_all[:], in_=xrn_in)

        stage = ctx.enter_context(tcx.tile_pool(name="stage", bufs=1))
        stage_r = ctx.enter_context(tcx.tile_pool(name="stage_r", bufs=3))
        ps_c = ctx.enter_context(tcx.tile_pool(name="ps_c", bufs=3, space="PSUM"))
        ps_neg = ctx.enter_context(tcx.tile_pool(name="ps_neg", bufs=1, space="PSUM"))
        ngb_ps = [ps_neg.tile([B, M], F32, tag=f"ngbp{i}", bufs=1, name=f"ngbp{i}")
                  for i in range(2)]
        negsb_pool = ctx.enter_context(tcx.tile_pool(name="negsb", bufs=1))
        negsb = [negsb_pool.tile([B, M], F32, tag=f"negsb{nt}", name=f"negsb{nt}")
                 for nt in range(NT)]
        cn_tiles = {}

        def emit_load(b):
            eng = nc.gpsimd if b % 2 == 0 else nc.scalar
            eng.dma_start(out=xTall[:, b], in_=xT_in[:, b])
            eng2 = nc.scalar if b % 2 == 0 else nc.gpsimd
            eng2.dma_start(out=ynTall[:, b], in_=ynT_in[:, b])

        def emit_B_mm0(nt, b):
            rows = min(PART, N - nt * PART)
            psc = ps_c.tile([PN, M], F32, tag="psc", name=f"psc_{nt}_{b}")
            nc.tensor.matmul(
                psc[:rows, :],
                xTall[:PD, b, 0, nt * PART:nt * PART + rows],
                ynTall[:PD, b, 0, :],
                start=True, stop=(DB == 1),
            )
            return psc

        def emit_B_rest(nt, b, psc):
            rows = min(PART, N - nt * PART)
            for db in range(1, DB):
                nc.tensor.matmul(
                    psc[:rows, :],
                    xTall[:PD, b, db, nt * PART:nt * PART + rows],
                    ynTall[:PD, b, db, :],
                    start=False, stop=(db == DB - 1),
                )
            if nt == 0:
                cn = stage.tile([PN, M], BF16, tag=f"cn0_{b}", name=f"cn_{nt}_{b}")
            else:
                cn = stage_r.tile([PN, M], BF16, tag="cn", name=f"cn_{nt}_{b}")
            cn_tiles[(nt, b)] = cn
            nc.scalar.activation(cn[:rows], psc[:rows], ACT.Copy,
                                 scale=xrn_all[:rows, b, nt:nt + 1], bias=1.0)
            eng = nc.scalar if b % 2 == 0 else nc.sync
            eng.dma_start(out=cost_stage[nt, b], in_=cn[:rows, :])

        def emit_ngb(nt, b):
            rows = min(PART, N - nt * PART)
            cn = cn_tiles.pop((nt, b))
            nc.tensor.matmul(
                ngb_ps[nt % 2][:, :],
                oneh[:rows, b, :],
                cn[:rows, :],
                start=(b == 0), stop=(b == B - 1),
                skip_group_check=True,
            )
            if b == B - 1:
                nc.scalar.copy(negsb[nt][:, :], ngb_ps[nt % 2][:, :])

        _Q = [nc.sync, nc.scalar, nc.gpsimd]

        def emit_hop2(nt, s0, s1):
            rows = min(PART, N - nt * PART)
            for s in range(s0, s1):
                src = cost_stage[nt, :, :, s * W:(s + 1) * W]
                eng = _Q[s % 3]
                eng.dma_start(
                    out=cost_strip[s * B:s * B + B,
                                   R * s + nt * PART:R * s + nt * PART + rows,
                                   0:W],
                    in_=src)

        # ---------------- prologue: loads + block-0 cost ----------------
        pscs = {}
        for b in range(B):
            emit_load(b)
        for b in range(B):
            pscs[b] = emit_B_mm0(0, b)
        for b in range(B):
            emit_B_rest(0, b, pscs.pop(b))
        emit_hop2(0, 0, S)

        # ---------------- DTW wavefront ----------------
        ps_carry = ctx.enter_context(tcx.tile_pool(name="ps_cr", bufs=1, space="PSUM"))
        mpool = ctx.enter_context(tcx.tile_pool(name="mpool", bufs=8))
        NCARRY = 4
        carry_all = ps_carry.tile([P, NCARRY, R], F32)

        def emit_carry(U, c0, c1):
            """Left-neighbor initials for superstep U+1 columns [c0:c1)."""
            base = R * U
            bnd = min(S - 1, U + 1) * B
            da = mpool.tile([P, R, 1], F32, tag="d", name=f"d_{U}_{c0}")
            # d[k] = min(last[base+k], last[base+k-1]); physical slot = logical+1
            nc.vector.tensor_tensor(
                da[0:bnd, 0:c1 - c0, :],
                tc_strip[0:bnd, base + c0 + 1:base + c1 + 1, SLOT - 1:SLOT],
                tc_strip[0:bnd, base + c0:base + c1, SLOT - 1:SLOT],
                OP.min,
            )
            ci = (U + 1) % NCARRY
            nc.tensor.matmul(
                carry_all[0:bnd + B, ci, c0:c1],
                shift8[0:bnd, 0:bnd + B],
                da[0:bnd, 0:c1 - c0, :],
                start=True, stop=False, skip_group_check=True,
            )
            nc.tensor.matmul(
                carry_all[0:bnd + B, ci, c0:c1],
                bigrow[0:1, 0:bnd + B],
                onesR[0:1, 0:c1 - c0],
                start=False, stop=True, skip_group_check=True,
            )
            return ci

        state = {"prev_carry": None, "out_lo": 0}

        def emit_stageC(U0, U1, sprinkle=None):
            for U in range(U0, U1):
                smax = min(S - 1, U)
                phi = (smax + 1) * B
                base = R * U
                for k in range(R):
                    q = base + k
                    mt_ = mpool.tile([P, W], F32, tag="m", name=f"m_{U}_{k}")
                    nc.vector.tensor_tensor(
                        mt_[0:phi, :],
                        tc_strip[0:phi, q, 0:W],
                        tc_strip[0:phi, q, 1:SLOT],
                        OP.min,
                    )
                    if U == 0:
                        init = 0.0 if k == 0 else BIG
                    else:
                        init = carry_all[0:phi, state["prev_carry"], k:k + 1]
                    nc.vector.tensor_tensor_scan(
                        tc_strip[0:phi, q + 1, 1:SLOT],
                        mt_[0:phi, :],
                        cost_strip[0:phi, q, :],
                        init,
                        OP.min,
                        OP.add,
                    )
                    if U + 1 < T_TOT:
                        if k == R - 2:
                            state["cps"] = emit_carry(U, 0, R - 1)
                        elif k == R - 1:
                            emit_carry(U, R - 1, R)
                            state["prev_carry"] = state["cps"]
                    if sprinkle:
                        for u_off, k_off, fn_ in sprinkle:
                            if U == U0 + u_off and k == k_off:
                                fn_()
                # stream finished slots out every 8 supersteps
                if (U + 1) % 8 == 0 and U + 1 < T_TOT:
                    lo_d, hi_d = state["out_lo"], (U + 1) * R
                    nc.sync.dma_start(out=tc_out[:, lo_d:hi_d, :],
                                      in_=tc_strip[:, lo_d + 1:hi_d + 1, :])
                    state["out_lo"] = hi_d

        # Sprinkle later blocks' cost work into the wavefront. Block nt is
        # first needed at superstep nt*UPB/R... = nt*16; emit it across the
        # preceding chunk. ngb(0) also lands in the first chunk.
        def mk(fn_, *a):
            return lambda: fn_(*a)

        def chunk_sprinkle(nt):
            spr = []
            for b in range(B):
                u = 2 * b
                spr.append((u, 3, mk(lambda n_, b_: pscs.__setitem__(
                    b_, emit_B_mm0(n_, b_)), nt, b)))
                spr.append((u, 7, mk(lambda n_, b_: emit_B_rest(
                    n_, b_, pscs.pop(b_)), nt, b)))
                # block nt's neg accumulation, one superstep after its cn
                spr.append((u + 1, 5, mk(emit_ngb, nt, b)))
                if nt == 1:
                    spr.append((u + 1, 1, mk(emit_ngb, 0, b)))
            spr.append((UPB - 2, 7, mk(emit_hop2, nt, 0, 8)))
            spr.append((UPB - 1, 7, mk(emit_hop2, nt, 8, S)))
            return spr

        for nt in range(1, NT):
            emit_stageC((nt - 1) * UPB, nt * UPB, sprinkle=chunk_sprinkle(nt))
        emit_stageC((NT - 1) * UPB, T_TOT)

        # ---------------- neg = logsumexp over m ----------------
        neg_pool = ctx.enter_context(tcx.tile_pool(name="negp", bufs=1))
        negsum = neg_pool.tile([B, M], F32)
        nc.vector.tensor_tensor(negsum[:], negsb[0][:, :], negsb[1][:, :], OP.add)
        for nt in range(2, NT):
            nc.vector.tensor_tensor(negsum[:], negsum[:], negsb[nt][:, :], OP.add)
        mx = neg_pool.tile([B, 1], F32)
        nc.vector.reduce_max(mx[:], negsum[:], AX.X)
        sh = neg_pool.tile([B, M], F32)
        nc.vector.tensor_scalar(sh[:], negsum[:], mx[:], None, OP.subtract)
        ex = neg_pool.tile([B, M], F32)
        esum = neg_pool.tile([B, 1], F32)
        nc.scalar.activation(ex[:], sh[:], ACT.Exp, accum_out=esum[:])
        lg = neg_pool.tile([B, 1], F32)
        nc.scalar.activation(lg[:], esum[:], ACT.Ln)
        negv = neg_pool.tile([B, 1], F32)
        nc.vector.tensor_add(negv[:], lg[:], mx[:])
        nc.sync.dma_start(out=neg_out[:, :], in_=negv[:])

        lo_d = state["out_lo"]
        nc.sync.dma_start(out=tc_out[:, lo_d:SLOTS, :],
                          in_=tc_strip[:, lo_d + 1:SLOTS + 1, :])

    import os
    n = _relax_same_engine_sems(nc) if not os.environ.get('KERNEL_NO_RELAX') else 0
    if os.environ.get("KERNEL_DEBUG"):
        print(f"relaxed {n} same-engine waits")
    nc.compile()
    return nc


# ---------------------------------------------------------------------------
# Host-side driver: sharding, layout prep, run, unskew, backtrack, final loss
# ---------------------------------------------------------------------------
import numpy as np
import ml_dtypes

BF16_NP = ml_dtypes.bfloat16
EPS = 1e-8

B_TOT, N_G, M_G, D_G = 64, 512, 512, 256
N_CORES = 8
B_LOC = B_TOT // N_CORES
S_G, W_G, R_G = 16, 32, 8
P_G = S_G * B_LOC
SLOTS_G = N_G + R_G * S_G
SLOT_G = W_G + 1
NT_G = N_G // 128
DB_G = D_G // 128

_NC_CACHE = {}


def _get_nc():
    if "nc" not in _NC_CACHE:
        _NC_CACHE["nc"] = _build_cfg(B=B_LOC, N=N_G, M=M_G, D=D_G,
                                     S=S_G, W=W_G, R=R_G)
    return _NC_CACHE["nc"]


def _unskew(tc_skew):
    tc = np.empty((B_LOC, N_G, M_G), np.float32)
    for s in range(S_G):
        for b in range(B_LOC):
            tc[b, :, s * W_G:(s + 1) * W_G] = \
                tc_skew[s * B_LOC + b, R_G * s:R_G * s + N_G, 1:SLOT_G]
    return tc


def _prep_core(x, y, ny_inv, nx_inv):
    """Device input layout for one core's batch slice.

    xT/ynT: [128, B_LOC, DB, N] bf16 with element [p,b,db,n] = t[b, n, db*128+p]
    xrn:    [128, B_LOC, NT] f32 = -1/||x_row|| arranged row-block-major.
    """
    yn = y * ny_inv[..., None]
    xt = np.ascontiguousarray(x.transpose(2, 0, 1))   # [D, B, N]
    ynt = np.ascontiguousarray(yn.transpose(2, 0, 1))
    xT = np.ascontiguousarray(
        xt.reshape(DB_G, 128, B_LOC, N_G).transpose(1, 2, 0, 3)).astype(BF16_NP)
    ynT = np.ascontiguousarray(
        ynt.reshape(DB_G, 128, B_LOC, M_G).transpose(1, 2, 0, 3)).astype(BF16_NP)
    xrn = np.ascontiguousarray(
        (-nx_inv).reshape(B_LOC, NT_G, 128).transpose(2, 0, 1)).astype(np.float32)
    return {"xT": xT, "ynT": ynT, "xrn": xrn}


def _host_finish(tc, x, y, neg):
    """Backtrack walk on the device tc + pos logsumexp (host side)."""
    Bt, Nn, Mm = tc.shape
    xn = x / np.maximum(np.linalg.norm(x, axis=-1, keepdims=True), EPS)
    yn = y / np.maximum(np.linalg.norm(y, axis=-1, keepdims=True), EPS)
    bidx = np.arange(Bt)
    i = np.full(Bt, Nn - 1, np.int64)
    j = np.full(Bt, Mm - 1, np.int64)
    Is, Js, Vs = [i.copy()], [j.copy()], [np.ones(Bt, bool)]
    active = (i > 0) & (j > 0)
    while active.any():
        a = tc[bidx, np.maximum(i - 1, 0), np.maximum(j - 1, 0)]
        bb = tc[bidx, np.maximum(i - 1, 0), j]
        c = tc[bidx, i, np.maximum(j - 1, 0)]
        diag = (a <= bb) & (a <= c)
        up = (~diag) & (bb <= c)
        ni = np.where(diag | up, i - 1, i)
        nj = np.where(diag | (~up), j - 1, j)
        i = np.where(active, ni, i)
        j = np.where(active, nj, j)
        Is.append(i.copy())
        Js.append(j.copy())
        Vs.append(active.copy())
        active = (i > 0) & (j > 0)
    at00 = (i == 0) & (j == 0)
    Is.append(np.zeros(Bt, np.int64))
    Js.append(np.zeros(Bt, np.int64))
    Vs.append(~at00)

    IS = np.stack(Is, 1)
    JS = np.stack(Js, 1)
    VS = np.stack(Vs, 1)
    costs = 1.0 - np.einsum("bld,bld->bl",
                            xn[bidx[:, None], IS], yn[bidx[:, None], JS])
    colsum = np.zeros((Bt, Mm), np.float32)
    np.add.at(colsum, (bidx[:, None], JS),
              np.where(VS, costs, 0.0).astype(np.float32))
    mxv = colsum.max(axis=1, keepdims=True)
    pos = (mxv + np.log(np.sum(np.exp(colsum - mxv),
                               axis=1, keepdims=True))).squeeze(1)
    return (pos.astype(np.float32) - neg).astype(np.float32)


def run_device(x, y, **kw):
    from concourse import bass_utils

    nc = _get_nc()
    nx_inv = 1.0 / np.maximum(np.linalg.norm(x, axis=-1), EPS)  # [B, N]
    ny_inv = 1.0 / np.maximum(np.linalg.norm(y, axis=-1), EPS)  # [B, M]
    in_maps = []
    for c in range(N_CORES):
        sl = slice(c * B_LOC, (c + 1) * B_LOC)
        in_maps.append(_prep_core(x[sl], y[sl], ny_inv[sl], nx_inv[sl]))
    res = bass_utils.run_bass_kernel_spmd(nc, in_maps, list(range(N_CORES)), **kw)
    tc = np.empty((B_TOT, N_G, M_G), np.float32)
    neg = np.empty(B_TOT, np.float32)
    for c in range(N_CORES):
        out = res.results[c]
        tc[c * B_LOC:(c + 1) * B_LOC] = _unskew(out["tc_out"])
        neg[c * B_LOC:(c + 1) * B_LOC] = out["neg_out"].reshape(B_LOC)
    return tc, neg, res


def kernel(x, y):
    x = np.asarray(x, dtype=np.float32)
    y = np.asarray(y, dtype=np.float32)
    tc, neg, _ = run_device(x, y)
    return _host_finish(tc, x, y, neg)
